# revision 4
# baseline (speedup 1.0000x reference)
"""Trainium2 Bass kernel for skipgram-style edge loss (embedding_lookup).

reference:
    u = emb[pos[:,0]]; v = emb[pos[:,1]]
    nu = emb[neg[...,0]]; nv = emb[neg[...,1]]
    loss = softplus(-<u,v>) + sum_k softplus(<nu_k,nv_k>)      # [E]

Strategy: replicate the table into each core's DRAM as bf16 (tolerance is
2e-2; bf16 quantization error on the loss is ~1e-4 absolute), split the
50k edge batch 8 ways.  Each core performs row gathers via SWDGE indirect
DMA (one 256B bf16 descriptor per embedding row).

The pairwise dot is restructured as  2<u,v> = sum((u+v)^2) - |u|^2 - |v|^2:
the second gather lands on top of the first with the SDMA CCE inline adder
(compute_op=add), so the DVE never does the elementwise multiply; the idle
ACT engine squares (u+v), and the DVE only does the d=128 segmented
reduction - two halving tensor_tensor adds at bf16 2x mode plus one short
1x tensor_reduce.  The |u|^2+|v|^2 correction per dot-slot is precomputed
on the host (it already packs the index tables) and subtracted on-chip.
All exp/ln work happens once at the end over the accumulated [P, NT*KSLOT]
dot buffer, keeping ACT table switches and the post-gather tail small:
loss = ln(prod_j(1+e^(s_j dot_j))) with a single Ln.

Task layout per core: edge e_local = (t*128 + p)*M + i maps to device
tile t, partition p, inner slot i; task j (0=pos, 1..5=neg) is the OUTER
slot dim (slot = j*M + i), so the pos/neg sign split is two contiguous
column ranges per tile.
"""

import ml_dtypes
import numpy as np

import concourse.bacc as bacc
import concourse.bass as bass
import concourse.mybir as mybir
from concourse.tile import TileContext
from concourse.bass_utils import run_bass_kernel_spmd

# Problem sizes (hardcoded per contract)
V = 500_000
D = 128
E = 50_000
K = 5

NCORES = 8
P = 128
J = K + 1                      # dot products per edge (1 pos + K neg)
EPC = E // NCORES              # 6250 edges per core
M = 7                          # edges per partition per tile
NT = -(-EPC // (P * M))        # 7 tiles per core
EPAD = NT * P * M              # 6272 padded edges per core
KSLOT = M * J                  # 42 dot slots per partition per tile

LAST_RESULTS = None            # BassKernelResults of the most recent run


def build_program(v=V, d=D, nt=NT, m=M, j=J, nchunks=3, emb_bufs=10):
    kslot = m * j
    nc = bacc.Bacc(trn_type="TRN2")
    emb = nc.dram_tensor("embeddings", [v, d], mybir.dt.bfloat16,
                         kind="ExternalInput")
    # [:, :nt*kslot] = left rows, [:, nt*kslot:] = right rows
    idx = nc.dram_tensor("idx", [P, 2 * nt * kslot], mybir.dt.int32,
                         kind="ExternalInput")
    # |u|^2 + |v|^2 per dot slot, same layout as the per-tile dot buffer
    nrm = nc.dram_tensor("nrm", [P, nt * kslot], mybir.dt.float32,
                         kind="ExternalInput")
    loss = nc.dram_tensor("loss", [P, nt * m], mybir.dt.float32,
                          kind="ExternalOutput")

    with TileContext(nc) as tc:
        with (
            tc.tile_pool(name="io", bufs=1) as io_pool,
            tc.tile_pool(name="emb", bufs=emb_bufs) as emb_pool,
            tc.tile_pool(name="small", bufs=3) as small_pool,
        ):
            # idx first: the gathers depend on it
            idx_sb = io_pool.tile([P, 2 * nt * kslot], mybir.dt.int32)
            nc.sync.dma_start(idx_sb[:], idx[:])
            nrm_sb = io_pool.tile([P, nt * kslot], mybir.dt.float32)
            nc.sync.dma_start(nrm_sb[:], nrm[:])
            idxl_sb = [idx_sb[:, t * kslot:(t + 1) * kslot]
                       for t in range(nt)]
            idxr_sb = [idx_sb[:, (nt + t) * kslot:(nt + t + 1) * kslot]
                       for t in range(nt)]

            # S = sum((u+v)^2) per dot slot, all tiles
            s_sb = io_pool.tile([P, nt * kslot], mybir.dt.float32)

            assert kslot % nchunks == 0
            csl = kslot // nchunks

            for t in range(nt):
                for c in range(nchunks):
                    lo = c * csl
                    el = emb_pool.tile([P, csl * d], mybir.dt.bfloat16,
                                       tag="el")
                    nc.gpsimd.indirect_dma_start(
                        out=el[:], out_offset=None, in_=emb[:],
                        in_offset=bass.IndirectOffsetOnAxis(
                            ap=idxl_sb[t][:, lo:lo + csl], axis=0))
                    # v-rows land on top of the u-rows through the SDMA
                    # CCE inline adder: el = u + v
                    nc.gpsimd.indirect_dma_start(
                        out=el[:], out_offset=None, in_=emb[:],
                        in_offset=bass.IndirectOffsetOnAxis(
                            ap=idxr_sb[t][:, lo:lo + csl], axis=0),
                        compute_op=mybir.AluOpType.add)
                    # ACT squares (idle engine), DVE reduces d=128
                    sq = small_pool.tile([P, csl * d], mybir.dt.bfloat16,
                                         tag="sq")
                    nc.scalar.square(sq[:], el[:])
                    h1 = small_pool.tile([P, csl * (d // 2)],
                                         mybir.dt.bfloat16, tag="h1")
                    sv = sq[:].rearrange("p (k two h) -> p k two h",
                                         two=2, h=d // 2)
                    nc.vector.tensor_add(h1[:], sv[:, :, 0, :], sv[:, :, 1, :])
                    h2 = small_pool.tile([P, csl * (d // 4)],
                                         mybir.dt.bfloat16, tag="h2")
                    hv = h1[:].rearrange("p (k two h) -> p k two h",
                                         two=2, h=d // 4)
                    nc.vector.tensor_add(h2[:], hv[:, :, 0, :], hv[:, :, 1, :])
                    nc.vector.reduce_sum(
                        s_sb[:, t * kslot + lo:t * kslot + lo + csl],
                        h2[:].rearrange("p (k h) -> p k h", h=d // 4),
                        axis=mybir.AxisListType.X)

            # 2*dot = S - (|u|^2+|v|^2); exp(s_j * dot) via ACT scale ±0.5
            dd = io_pool.tile([P, nt * kslot], mybir.dt.float32)
            nc.vector.tensor_sub(dd[:], s_sb[:], nrm_sb[:])
            ex = io_pool.tile([P, nt * kslot], mybir.dt.float32)
            ddv = dd[:].rearrange("p (t j i) -> p t j i", t=nt, j=j)
            exv = ex[:].rearrange("p (t j i) -> p t j i", t=nt, j=j)
            nc.scalar.activation(exv[:, :, 0, :], ddv[:, :, 0, :],
                                 mybir.ActivationFunctionType.Exp,
                                 scale=-0.5)
            nc.scalar.activation(exv[:, :, 1:, :], ddv[:, :, 1:, :],
                                 mybir.ActivationFunctionType.Exp,
                                 scale=0.5)
            nc.vector.tensor_scalar_add(ex[:], ex[:], 1.0)
            # product over the J tasks of each edge: [P, (t j i)] ->
            # fold j (6 = 2*3): pairwise then triple
            assert j == 6
            b = io_pool.tile([P, nt * 3 * m], mybir.dt.float32)
            bv = b[:].rearrange("p (t j i) -> p t j i", t=nt, j=3)
            nc.vector.tensor_mul(bv[:], exv[:, :, :3, :], exv[:, :, 3:, :])
            cc = io_pool.tile([P, nt * m], mybir.dt.float32)
            ccv = cc[:].rearrange("p (t i) -> p t i", t=nt)
            nc.vector.tensor_mul(ccv[:], bv[:, :, 0, :], bv[:, :, 1, :])
            loss_sb = io_pool.tile([P, nt * m], mybir.dt.float32)
            lv = loss_sb[:].rearrange("p (t i) -> p t i", t=nt)
            nc.vector.tensor_mul(lv[:], ccv[:], bv[:, :, 2, :])
            nc.scalar.activation(loss_sb[:], loss_sb[:],
                                 mybir.ActivationFunctionType.Ln)
            nc.sync.dma_start(loss[:], loss_sb[:])
    nc.finalize()
    return nc


def _pack_indices(pos_edges, neg_edges, core):
    """[P, 2*NT*KSLOT] int32 row indices for one core (left cols, right
    cols)."""
    lo = core * EPC
    hi = lo + EPC
    tl = np.zeros((EPAD, J), np.int32)
    tr = np.zeros((EPAD, J), np.int32)
    tl[:EPC, 0] = pos_edges[lo:hi, 0]
    tl[:EPC, 1:] = neg_edges[lo:hi, :, 0]
    tr[:EPC, 0] = pos_edges[lo:hi, 1]
    tr[:EPC, 1:] = neg_edges[lo:hi, :, 1]
    # [EPAD, J] -> [NT, P, M, J] -> [P, NT, J, M] -> [P, NT*KSLOT]
    il = tl.reshape(NT, P, M, J).transpose(1, 0, 3, 2).reshape(P, NT * KSLOT)
    ir = tr.reshape(NT, P, M, J).transpose(1, 0, 3, 2).reshape(P, NT * KSLOT)
    return il, ir


_PROGRAM = None


def kernel(embeddings, pos_edges, neg_edges):
    global _PROGRAM, LAST_RESULTS
    emb_bf16 = np.ascontiguousarray(
        np.asarray(embeddings, dtype=np.float32).astype(ml_dtypes.bfloat16))
    pos_edges = np.asarray(pos_edges).astype(np.int32)
    neg_edges = np.asarray(neg_edges).astype(np.int32)

    # per-row squared norms of the bf16 table (so the correction cancels
    # exactly against what the device computes from bf16 rows)
    ef = emb_bf16.astype(np.float32)
    norms = np.einsum("ij,ij->i", ef, ef).astype(np.float32)

    if _PROGRAM is None:
        _PROGRAM = build_program()
    nc = _PROGRAM

    in_maps = []
    for c in range(NCORES):
        il, ir = _pack_indices(pos_edges, neg_edges, c)
        in_maps.append({
            "embeddings": emb_bf16,
            "idx": np.ascontiguousarray(np.concatenate([il, ir], axis=1)),
            "nrm": np.ascontiguousarray(norms[il] + norms[ir]),
        })

    res = run_bass_kernel_spmd(nc, in_maps, core_ids=list(range(NCORES)))
    LAST_RESULTS = res

    out = np.empty(E, np.float32)
    for c in range(NCORES):
        dev = np.asarray(res.results[c]["loss"], np.float32)  # [P, NT*M]
        ordered = dev.reshape(P, NT, M).transpose(1, 0, 2).reshape(EPAD)
        out[c * EPC:(c + 1) * EPC] = ordered[:EPC]
    return out


# revision 5
# speedup vs baseline: 1.5851x; 1.5851x over previous
"""Trainium2 Bass kernel for skipgram-style edge loss (embedding_lookup).

reference:
    u = emb[pos[:,0]]; v = emb[pos[:,1]]
    nu = emb[neg[...,0]]; nv = emb[neg[...,1]]
    loss = softplus(-<u,v>) + sum_k softplus(<nu_k,nv_k>)      # [E]

Strategy: replicate the table into each core's DRAM as bf16 (tolerance is
2e-2; bf16 quantization error on the loss is ~1e-4 absolute), split the
50k edge batch 8 ways.  Each core performs row gathers via SWDGE indirect
DMA (one 256B bf16 descriptor per embedding row - half the HBM traffic
and half the per-descriptor SDMA beat count vs f32).  One gather call per
tile side (not per sub-chunk): Q7 descriptor emission paces the SDMA
engines, and each INDIRECT1D instruction costs a fixed ~310ns sequencer
gap on top of ~0.61ns/row emission, so fewer+bigger calls keep the SDMA
queue fed.  DVE does the pairwise mul at bf16 2x mode, then reduces d=128
with two halving tensor_tensor adds (2x mode) plus one short 1x
tensor_reduce (tensor_reduce has no 2x uop).  ACT applies exp with the
pos-edge sign flip folded into the activation scale; softplus is computed
as ln(prod_j(1+e^x_j)) with one final Ln.

Task layout per core: edge e_local = (t*128 + p)*M + i maps to device
tile t, partition p, inner slot i; task j (0=pos, 1..5=neg) is the OUTER
slot dim (slot = j*M + i), so the pos/neg sign split is two contiguous
column ranges.  idx DRAM layout puts tile 0's left+right columns first so
a small head DMA unblocks the first gather ~2us earlier.
"""

import ml_dtypes
import numpy as np

import concourse.bacc as bacc
import concourse.bass as bass
import concourse.mybir as mybir
from concourse.tile import TileContext
from concourse.bass_utils import run_bass_kernel_spmd

# Problem sizes (hardcoded per contract)
V = 500_000
D = 128
E = 50_000
K = 5

NCORES = 8
P = 128
J = K + 1                      # dot products per edge (1 pos + K neg)
EPC = E // NCORES              # 6250 edges per core
M = 7                          # edges per partition per tile
NT = -(-EPC // (P * M))        # 7 tiles per core
EPAD = NT * P * M              # 6272 padded edges per core
KSLOT = M * J                  # 42 dot slots per partition per tile

LAST_RESULTS = None            # BassKernelResults of the most recent run


def build_program(v=V, d=D, nt=NT, m=M, j=J, emb_bufs=4):
    kslot = m * j
    nc = bacc.Bacc(trn_type="TRN2")
    emb = nc.dram_tensor("embeddings", [v, d], mybir.dt.bfloat16,
                         kind="ExternalInput")
    # col layout: [t0_l, t0_r, t1_l, t1_r, ..., t6_l, t6_r]
    idx = nc.dram_tensor("idx", [P, 2 * nt * kslot], mybir.dt.int32,
                         kind="ExternalInput")
    loss = nc.dram_tensor("loss", [P, nt * m], mybir.dt.float32,
                          kind="ExternalOutput")

    with TileContext(nc) as tc:
        with (
            tc.tile_pool(name="io", bufs=1) as io_pool,
            tc.tile_pool(name="emb", bufs=emb_bufs) as emb_pool,
            tc.tile_pool(name="small", bufs=3) as small_pool,
        ):
            loss_sb = io_pool.tile([P, nt * m], mybir.dt.float32)

            # idx split: tile 0's columns first (small, fast), rest behind
            idx_sb = io_pool.tile([P, 2 * nt * kslot], mybir.dt.int32)
            nc.sync.dma_start(idx_sb[:, :2 * kslot], idx[:, :2 * kslot])
            nc.sync.dma_start(idx_sb[:, 2 * kslot:], idx[:, 2 * kslot:])
            idxl_sb = [idx_sb[:, 2 * t * kslot:(2 * t + 1) * kslot]
                       for t in range(nt)]
            idxr_sb = [idx_sb[:, (2 * t + 1) * kslot:(2 * t + 2) * kslot]
                       for t in range(nt)]

            for t in range(nt):
                dots = small_pool.tile([P, kslot], mybir.dt.float32,
                                       tag="dots")
                el = emb_pool.tile([P, kslot * d], mybir.dt.bfloat16,
                                   tag="el")
                er = emb_pool.tile([P, kslot * d], mybir.dt.bfloat16,
                                   tag="er")
                nc.gpsimd.indirect_dma_start(
                    out=el[:], out_offset=None, in_=emb[:],
                    in_offset=bass.IndirectOffsetOnAxis(
                        ap=idxl_sb[t][:], axis=0))
                nc.gpsimd.indirect_dma_start(
                    out=er[:], out_offset=None, in_=emb[:],
                    in_offset=bass.IndirectOffsetOnAxis(
                        ap=idxr_sb[t][:], axis=0))
                # pairwise mul at bf16 2x mode, in place
                nc.vector.tensor_mul(el[:], el[:], er[:])
                # d=128 -> 64 -> 32 via 2x-mode adds, then 1x reduce(32)
                h1 = small_pool.tile([P, kslot * (d // 2)],
                                     mybir.dt.bfloat16, tag="h1")
                pv = el[:].rearrange("p (k two h) -> p k two h",
                                     two=2, h=d // 2)
                nc.vector.tensor_add(h1[:], pv[:, :, 0, :], pv[:, :, 1, :])
                h2 = small_pool.tile([P, kslot * (d // 4)],
                                     mybir.dt.bfloat16, tag="h2")
                hv = h1[:].rearrange("p (k two h) -> p k two h",
                                     two=2, h=d // 4)
                nc.vector.tensor_add(h2[:], hv[:, :, 0, :], hv[:, :, 1, :])
                nc.vector.reduce_sum(
                    dots[:],
                    h2[:].rearrange("p (k h) -> p k h", h=d // 4),
                    axis=mybir.AxisListType.X)

                # ln(prod_j (1 + exp(s_j dot_j))) via exp + product tree
                ex = small_pool.tile([P, kslot], mybir.dt.float32, tag="ex")
                nc.scalar.activation(ex[:, :m], dots[:, :m],
                                     mybir.ActivationFunctionType.Exp,
                                     scale=-1.0)
                nc.scalar.activation(ex[:, m:], dots[:, m:],
                                     mybir.ActivationFunctionType.Exp,
                                     scale=1.0)
                nc.vector.tensor_scalar_add(ex[:], ex[:], 1.0)
                assert j == 6
                b = small_pool.tile([P, 3 * m], mybir.dt.float32, tag="b")
                cc = small_pool.tile([P, m], mybir.dt.float32, tag="c")
                nc.vector.tensor_mul(b[:], ex[:, :3 * m], ex[:, 3 * m:])
                nc.vector.tensor_mul(cc[:], b[:, :m], b[:, m:2 * m])
                nc.vector.tensor_mul(loss_sb[:, t * m:(t + 1) * m],
                                     cc[:], b[:, 2 * m:])

            nc.scalar.activation(loss_sb[:], loss_sb[:],
                                 mybir.ActivationFunctionType.Ln)
            nc.sync.dma_start(loss[:], loss_sb[:])
    nc.finalize()
    return nc


def _pack_indices(pos_edges, neg_edges, core):
    """[P, 2*NT*KSLOT] int32 row indices, tile-interleaved [t0_l, t0_r,
    t1_l, t1_r, ...]."""
    lo = core * EPC
    hi = lo + EPC
    tl = np.zeros((EPAD, J), np.int32)
    tr = np.zeros((EPAD, J), np.int32)
    tl[:EPC, 0] = pos_edges[lo:hi, 0]
    tl[:EPC, 1:] = neg_edges[lo:hi, :, 0]
    tr[:EPC, 0] = pos_edges[lo:hi, 1]
    tr[:EPC, 1:] = neg_edges[lo:hi, :, 1]
    # [EPAD, J] -> [NT, P, M, J] -> [P, NT, J, M] -> [P, NT, KSLOT]
    il = tl.reshape(NT, P, M, J).transpose(1, 0, 3, 2).reshape(P, NT, KSLOT)
    ir = tr.reshape(NT, P, M, J).transpose(1, 0, 3, 2).reshape(P, NT, KSLOT)
    # interleave: [P, NT, 2, KSLOT] -> [P, 2*NT*KSLOT]
    packed = np.stack([il, ir], axis=2).reshape(P, 2 * NT * KSLOT)
    return np.ascontiguousarray(packed)


_PROGRAM = None


def kernel(embeddings, pos_edges, neg_edges):
    global _PROGRAM, LAST_RESULTS
    emb_bf16 = np.ascontiguousarray(
        np.asarray(embeddings, dtype=np.float32).astype(ml_dtypes.bfloat16))
    pos_edges = np.asarray(pos_edges).astype(np.int32)
    neg_edges = np.asarray(neg_edges).astype(np.int32)

    if _PROGRAM is None:
        _PROGRAM = build_program()
    nc = _PROGRAM

    in_maps = [
        {"embeddings": emb_bf16,
         "idx": _pack_indices(pos_edges, neg_edges, c)}
        for c in range(NCORES)
    ]

    res = run_bass_kernel_spmd(nc, in_maps, core_ids=list(range(NCORES)))
    LAST_RESULTS = res

    out = np.empty(E, np.float32)
    for c in range(NCORES):
        dev = np.asarray(res.results[c]["loss"], np.float32)  # [P, NT*M]
        ordered = dev.reshape(P, NT, M).transpose(1, 0, 2).reshape(EPAD)
        out[c * EPC:(c + 1) * EPC] = ordered[:EPC]
    return out
